# revision 5
# baseline (speedup 1.0000x reference)
"""DiffAE attention block (GroupNorm -> qkv 1x1conv -> attention -> proj -> residual)
as a Bass/Tile kernel on 8 TRN2 NeuronCores.

Sharding: data-parallel over batch. B=32 samples, 4 per core. Attention is
per-sample, so no collectives are needed: inputs are sharded host-side and
outputs gathered host-side.

Math restructure vs the straightforward reference:
  * proj is folded into the v weights host-side: W' = proj_w @ wv, and since
    the per-column softmax scale commutes with the channel projection and
    softmax rows sum to exactly 1 against the kernel's own denominator, the
    v/proj biases collapse to a constant output bias bo = proj_w @ bv + pb.
    This removes the whole proj matmul stage (12.5% of the FLOPs).
  * All four remaining matmul stages (q, k, scores, attn@v) run in fp8-e4m3
    with perf_mode=DoubleRow (K=256 per instruction, ~1.4x bf16 TensorE rate).
    fp32 accumulation in PSUM throughout.
  * fp8 scaling: TRN e4m3 spans [2^-9, 240]. GroupNorm output h is stored
    x8, q/k weights x4 (so q,k tiles are 32x), W' x4 (v tiles 32x). Scores
    PSUM = 1024 x true score; exp applies scale SCALE/1024 and bias -2 so the
    largest exp value stays ~40 << 240 (softmax shift-invariance cancels the
    -2 against the denominator). The ones matrix for the denominator
    partition-broadcast carries 32.0 so rs = 1/(32*sum e) exactly cancels the
    32x in the v tiles.

Per-core layout (all [channel, spatial] "c,n" unless noted):
  x          4 tiles [128, 1024] f32 per sample
  h, q, k    [128, CT=4, 1024] fp8 super-tiles (channel-tile-major free dim)
  vT         [128, MT=8, 512] fp8 (spatial-major partitions, channel free)
  e          [128, MT=8, 512] fp8 per 512-column chunk; esum via DVE adds,
             partition-broadcast via a 32.0-matmul, fast reciprocal
  out        (v' @ e^T) * rs + bo + x, two DVE ops per tile, DMA to DRAM
"""

import numpy as np
import ml_dtypes

import concourse.bacc as bacc
import concourse.bass as bass
import concourse.mybir as mybir
import concourse.tile as tile
from concourse import bass_isa
from concourse.bass_utils import run_bass_kernel_spmd

N_CORES = 8
B, C, H, W = 32, 512, 32, 32
HW = H * W                      # 1024 spatial positions
BS = B // N_CORES               # 4 samples per core
GROUPS = 32
EPS = 1e-5
SCALE = float(C) ** -0.5
P = 128
CT = C // P                     # 4 channel tiles
MT = HW // P                    # 8 spatial tiles
KP = CT // 2                    # 2 DoubleRow contraction pairs over channels
MP = MT // 2                    # 4 DoubleRow contraction pairs over spatial
NF = 512                        # matmul moving-dim chunk (output columns)
NCH = HW // NF                  # 2 column chunks
F32 = mybir.dt.float32
F32R = mybir.dt.float32r
BF16 = mybir.dt.bfloat16
F8 = mybir.dt.float8e4
AX = mybir.AxisListType
ALU = mybir.AluOpType
ACTF = mybir.ActivationFunctionType
DR = mybir.MatmulPerfMode.DoubleRow

H_SC = 8.0                      # h stored as 8*h
W_SC = 4.0                      # q/k/v weights stored as 4*W
QK_SC = H_SC * W_SC             # q,k tiles are 32x true
S_SC = QK_SC * QK_SC            # scores PSUM is 1024x true
E_BIAS = -2.0                   # exp(s - 2): keeps max e ~40 << 240 (fp8 max)
ONE_V = 32.0                    # denominator matmul constant; 1/(32 sum e)
                                # cancels the 32x in the v tiles


def build():
    nc = bacc.Bacc("TRN2", target_bir_lowering=False, debug=False,
                   num_devices=N_CORES, num_swdge_queues=4)

    x_d = nc.declare_dram_parameter("x", [BS, C, HW], F32, isOutput=False)
    wq_d = nc.declare_dram_parameter("wq", [P, KP, 2, C], F8, isOutput=False)
    wk_d = nc.declare_dram_parameter("wk", [P, KP, 2, C], F8, isOutput=False)
    wv_d = nc.declare_dram_parameter("wv", [P, KP, 2, C], F8, isOutput=False)
    gm_d = nc.declare_dram_parameter("gm", [P, CT, GROUPS], F32R, isOutput=False)
    gmT_d = nc.declare_dram_parameter("gmT", [GROUPS, C], F32R, isOutput=False)
    bq_d = nc.declare_dram_parameter("bq", [P, CT], F32, isOutput=False)
    bk_d = nc.declare_dram_parameter("bk", [P, CT], F32, isOutput=False)
    bo_d = nc.declare_dram_parameter("bo", [P, CT], F32, isOutput=False)
    gnw_d = nc.declare_dram_parameter("gnw", [P, CT], F32, isOutput=False)
    gnb_d = nc.declare_dram_parameter("gnb", [P, CT], F32, isOutput=False)
    out_d = nc.declare_dram_parameter("out", [BS, C, HW], F32, isOutput=True)

    with tile.TileContext(nc) as tc:
        build_tile(tc, x_d, wq_d, wk_d, wv_d, gm_d, gmT_d,
                   bq_d, bk_d, bo_d, gnw_d, gnb_d, out_d)
    nc.finalize()
    return nc


def build_tile(tc, x_d, wq_d, wk_d, wv_d, gm_d, gmT_d,
               bq_d, bk_d, bo_d, gnw_d, gnb_d, out_d):
    nc = tc.nc
    from contextlib import ExitStack
    with ExitStack() as ctx:
        ctx.enter_context(nc.allow_low_precision(
            reason="fp8 DoubleRow matmuls; fp32 accumulate in PSUM"))
        consts = ctx.enter_context(tc.tile_pool(name="consts", bufs=1))
        xs = ctx.enter_context(tc.tile_pool(name="xs", bufs=8))
        hp = ctx.enter_context(tc.tile_pool(name="hp", bufs=2))
        qp = ctx.enter_context(tc.tile_pool(name="qp", bufs=2))
        kp_ = ctx.enter_context(tc.tile_pool(name="kp", bufs=2))
        vp = ctx.enter_context(tc.tile_pool(name="vp", bufs=2))
        ep = ctx.enter_context(tc.tile_pool(name="ep", bufs=4))
        rp = ctx.enter_context(tc.tile_pool(name="rp", bufs=3))
        sdp = ctx.enter_context(tc.tile_pool(name="sdp", bufs=4))
        op = ctx.enter_context(tc.tile_pool(name="op", bufs=10))
        st = ctx.enter_context(tc.tile_pool(name="st", bufs=16))
        pgn = ctx.enter_context(tc.tile_pool(name="pgn", bufs=2, space="PSUM"))
        pmm = ctx.enter_context(tc.tile_pool(name="pmm", bufs=6, space="PSUM"))

        # ---- constants / weights (small ones first so the first sample's
        # GroupNorm isn't stuck behind the weight transfers) ----
        gm_sb = consts.tile([P, CT, GROUPS], F32R, name="gm_sb")
        gmT_sb = consts.tile([GROUPS, C], F32R, name="gmT_sb")
        bq_sb = consts.tile([P, CT], F32, name="bq_sb")
        bk_sb = consts.tile([P, CT], F32, name="bk_sb")
        bo_sb = consts.tile([P, CT], F32, name="bo_sb")
        gnw_sb = consts.tile([P, CT], F32, name="gnw_sb")
        gnb_sb = consts.tile([P, CT], F32, name="gnb_sb")
        epsg_sb = consts.tile([GROUPS, 1], F32, name="epsg_sb")
        ebias_sb = consts.tile([P, 1], F32, name="ebias_sb")
        ones_sb = consts.tile([P, P], BF16, name="ones_sb")
        wq_sb = consts.tile([P, KP, 2, C], F8, name="wq_sb")
        wk_sb = consts.tile([P, KP, 2, C], F8, name="wk_sb")
        wv_sb = consts.tile([P, KP, 2, C], F8, name="wv_sb")

        for sb, d in ((gm_sb, gm_d), (gmT_sb, gmT_d),
                      (bq_sb, bq_d), (bk_sb, bk_d), (bo_sb, bo_d),
                      (gnw_sb, gnw_d), (gnb_sb, gnb_d)):
            nc.gpsimd.dma_start(out=sb, in_=d[:])
        nc.vector.memset(epsg_sb, EPS)
        nc.vector.memset(ebias_sb, E_BIAS)
        nc.vector.memset(ones_sb, ONE_V)
        # preload the sqrt ACT table so sample 0's rstd chain doesn't pay
        # the table-load latency on the critical path
        sqwarm = st.tile([GROUPS, 1], F32, name="sqwarm", tag="sqwarm")
        nc.scalar.activation(out=sqwarm, in_=epsg_sb, func=ACTF.Sqrt)

        inv_gsz = 1.0 / (C // GROUPS * HW)

        def prep_load(s):
            """x DMA for sample s."""
            x_t = []
            for ct in range(CT):
                xt = xs.tile([P, HW], F32, name=f"x_s{s}_{ct}", tag="x")
                for n in range(NCH):
                    nsl = slice(n * NF, (n + 1) * NF)
                    nc.sync.dma_start(out=xt[:, nsl],
                                      in_=x_d[s, ct * P:(ct + 1) * P, nsl])
                x_t.append(xt)
            if s == 0:
                # weights go after the first x so sample 0's stats start
                # sooner; they are small in fp8 (256KB each)
                for sb, d in ((wq_sb, wq_d), (wk_sb, wk_d), (wv_sb, wv_d)):
                    nc.gpsimd.dma_start(out=sb, in_=d[:])
            return x_t

        def prep_stats(s, x_t):
            """GroupNorm for sample s -> h super-tile (fp8, 8x scaled)."""
            gsum = pgn.tile([GROUPS, 2], F32, name=f"gsum_{s}", tag="ps")
            bnsts = []
            for ct in range(CT):
                bnst = st.tile([P, NCH, 6], F32, name=f"bnst_{s}_{ct}",
                               tag=f"bnst{ct}")
                xv = x_t[ct].rearrange("p (a b) -> p a b", b=NF)
                for sg in range(NCH):
                    nc.vector.bn_stats(bnst[:, sg, :], xv[:, sg, :])
                bnsts.append(bnst)
            for ct in range(CT):
                rowmv = st.tile([P, 2], F32, name=f"rowmv_{s}_{ct}",
                                tag="rowmv")
                nc.vector.bn_aggr(rowmv, bnsts[ct])
                # me = [mean, E[x^2]] * HW (uniform scale folds into inv_gsz)
                me = st.tile([P, 2], F32R, name=f"me_{s}_{ct}", tag="me")
                nc.vector.tensor_scalar_mul(me[:, 0:1], rowmv[:, 0:1],
                                            float(HW))
                nc.vector.scalar_tensor_tensor(
                    out=me[:, 1:2], in0=rowmv[:, 0:1],
                    scalar=rowmv[:, 0:1], in1=rowmv[:, 1:2],
                    op0=ALU.mult, op1=ALU.add)
                nc.vector.tensor_scalar_mul(me[:, 1:2], me[:, 1:2],
                                            float(HW))
                nc.tensor.matmul(gsum, lhsT=gm_sb[:, ct, :], rhs=me,
                                 start=(ct == 0), stop=(ct == CT - 1))

            # mv[:,0] = mean, mv[:,1] = 1/sqrt(var+eps)
            mv = st.tile([GROUPS, 2], F32R, name=f"mv_{s}", tag="mv")
            nc.scalar.mul(out=mv[:, 0:1], in_=gsum[:, 0:1], mul=inv_gsz)
            ex2 = st.tile([GROUPS, 1], F32, name=f"ex2_{s}", tag="ex2")
            nc.scalar.mul(out=ex2, in_=gsum[:, 1:2], mul=inv_gsz)
            msq = st.tile([GROUPS, 1], F32, name=f"msq_{s}", tag="msq")
            nc.vector.tensor_mul(msq, mv[:, 0:1], mv[:, 0:1])
            var = st.tile([GROUPS, 1], F32, name=f"var_{s}", tag="var")
            nc.vector.tensor_sub(var, ex2, msq)
            nc.scalar.activation(out=var, in_=var, func=ACTF.Sqrt,
                                 bias=epsg_sb, scale=1.0)
            nc.vector.reciprocal(out=mv[:, 1:2], in_=var)

            # expand per-group (mean, rstd) to per-channel alpha/beta;
            # gnw/gnb arrive pre-scaled by 8 so h is stored as 8*h
            ht = hp.tile([P, CT, HW], F8, name=f"h_{s}", tag="h")
            for ct in range(CT):
                eps_ps = pgn.tile([P, 2], F32, name=f"exp_{s}_{ct}", tag="ps")
                nc.tensor.matmul(eps_ps, lhsT=gmT_sb[:, ct * P:(ct + 1) * P],
                                 rhs=mv, start=True, stop=True)
                exs = st.tile([P, 2], F32, name=f"exs_{s}_{ct}", tag="exs")
                nc.vector.tensor_copy(exs, eps_ps)
                alpha = st.tile([P, 1], F32, name=f"al_{s}_{ct}", tag="al")
                nc.vector.tensor_mul(alpha, gnw_sb[:, ct:ct + 1], exs[:, 1:2])
                mal = st.tile([P, 1], F32, name=f"mal_{s}_{ct}", tag="mal")
                nc.vector.tensor_mul(mal, exs[:, 0:1], alpha)
                beta = st.tile([P, 1], F32, name=f"be_{s}_{ct}", tag="be")
                nc.vector.tensor_sub(beta, gnb_sb[:, ct:ct + 1], mal)
                nc.vector.tensor_scalar(out=ht[:, ct, :], in0=x_t[ct],
                                        scalar1=alpha, scalar2=beta,
                                        op0=ALU.mult, op1=ALU.add)
            return ht

        def body_qkv(s, h_t):
            """q, k (channel-major) and vT (spatial-major) for sample s."""
            q_t = qp.tile([P, CT, HW], F8, name=f"q_{s}", tag="q")
            k_t = kp_.tile([P, CT, HW], F8, name=f"k_{s}", tag="k")
            for w_sb, b_sb, dst, tag in ((wq_sb, bq_sb, q_t, "q"),
                                         (wk_sb, bk_sb, k_t, "k")):
                for mt in range(CT):
                    ps_n = [pmm.tile([P, NF], F32,
                                     name=f"{tag}p_{s}_{mt}_{n}", tag="ps")
                            for n in range(NCH)]
                    for kpi in range(KP):
                        lhsT = w_sb[:, kpi, :, mt * P:(mt + 1) * P]
                        for n in range(NCH):
                            nsl = slice(n * NF, (n + 1) * NF)
                            nc.tensor.matmul(
                                ps_n[n], lhsT=lhsT,
                                rhs=h_t[:, 2 * kpi:2 * kpi + 2, nsl],
                                start=(kpi == 0), stop=(kpi == KP - 1),
                                perf_mode=DR)
                    for n in range(NCH):
                        nsl = slice(n * NF, (n + 1) * NF)
                        nc.scalar.activation(out=dst[:, mt, nsl],
                                             in_=ps_n[n], func=ACTF.Identity,
                                             bias=b_sb[:, mt:mt + 1],
                                             scale=1.0)

            vT = vp.tile([P, MT, C], F8, name=f"v_{s}", tag="v")
            for mt in range(MT):
                ps = pmm.tile([P, C], F32, name=f"vp_{s}_{mt}", tag="ps")
                for kpi in range(KP):
                    nc.tensor.matmul(
                        ps, lhsT=h_t[:, 2 * kpi:2 * kpi + 2,
                                     mt * P:(mt + 1) * P],
                        rhs=wv_sb[:, kpi, :, :],
                        start=(kpi == 0), stop=(kpi == KP - 1),
                        perf_mode=DR)
                nc.vector.tensor_copy(vT[:, mt, :], ps)
            return q_t, k_t, vT

        def attn_scores(s, n, q_t, k_t):
            """scores + exp + softmax denominator for column chunk n."""
            nsl = slice(n * NF, (n + 1) * NF)
            e_t = ep.tile([P, MT, NF], F8, name=f"e_{s}_{n}", tag="e")
            esum = sdp.tile([P, NF], BF16, name=f"esum_{s}_{n}", tag="esum")
            for mt in range(MT):
                ps = pmm.tile([P, NF], F32, name=f"ep_{s}_{n}_{mt}", tag="ps")
                for kpi in range(KP):
                    nc.tensor.matmul(
                        ps, lhsT=k_t[:, 2 * kpi:2 * kpi + 2,
                                     mt * P:(mt + 1) * P],
                        rhs=q_t[:, 2 * kpi:2 * kpi + 2, nsl],
                        start=(kpi == 0), stop=(kpi == KP - 1),
                        perf_mode=DR)
                nc.scalar.activation(out=e_t[:, mt, :], in_=ps, func=ACTF.Exp,
                                     scale=SCALE / S_SC, bias=ebias_sb)
                # softmax denominator: running elementwise sum of e (DVE)
                if mt == 1:
                    nc.vector.tensor_add(esum, e_t[:, 0, :], e_t[:, 1, :])
                elif mt > 1:
                    nc.vector.tensor_add(esum, esum, e_t[:, mt, :])
            return e_t, esum

        def attn_denom(s, n, esum):
            """partition-sum broadcast (x32) + fast reciprocal -> rs."""
            sb_ps = pmm.tile([P, NF], F32, name=f"sb_{s}_{n}", tag="ps")
            nc.tensor.matmul(sb_ps, lhsT=ones_sb, rhs=esum,
                             start=True, stop=True)
            rs = rp.tile([P, NF], F32, name=f"rs_{s}_{n}", tag="rs")
            nc.vector.reciprocal_approx_fast(out=rs, in_=sb_ps)
            return rs

        def attn_out(s, n, x_t, vT, e_t, rs):
            """(v' @ e^T) * rs + bo + x, store for column chunk n."""
            nsl = slice(n * NF, (n + 1) * NF)
            for ct in range(CT):
                ps = pmm.tile([P, NF], F32, name=f"h2p_{s}_{n}_{ct}",
                              tag="ps")
                for mp in range(MP):
                    nc.tensor.matmul(
                        ps, lhsT=vT[:, 2 * mp:2 * mp + 2,
                                    ct * P:(ct + 1) * P],
                        rhs=e_t[:, 2 * mp:2 * mp + 2, :],
                        start=(mp == 0), stop=(mp == MP - 1),
                        perf_mode=DR)
                t_sb = op.tile([P, NF], F32, name=f"t_{s}_{n}_{ct}", tag="t")
                nc.vector.tensor_mul(t_sb, ps, rs)
                o_sb = op.tile([P, NF], F32, name=f"o_{s}_{n}_{ct}", tag="o")
                nc.vector.scalar_tensor_tensor(
                    out=o_sb, in0=t_sb, scalar=bo_sb[:, ct:ct + 1],
                    in1=x_t[ct][:, nsl], op0=ALU.add, op1=ALU.add)
                nc.sync.dma_start(
                    out=out_d[s, ct * P:(ct + 1) * P, nsl], in_=o_sb)

        # software pipeline: sample s+1's x load + GroupNorm stats chain
        # are emitted inside sample s's body so the serial stats chain hides
        # under the qkv/attention matmul burst
        x0 = prep_load(0)
        h0 = prep_stats(0, x0)
        cur = (x0, h0)
        for s in range(BS):
            x_t, h_t = cur
            nxt_x = prep_load(s + 1) if s + 1 < BS else None
            q_t, k_t, vT = body_qkv(s, h_t)
            nxt = (nxt_x, prep_stats(s + 1, nxt_x)) if s + 1 < BS else None
            e0, esum0 = attn_scores(s, 0, q_t, k_t)
            e1, esum1 = attn_scores(s, 1, q_t, k_t)
            rs0 = attn_denom(s, 0, esum0)
            attn_out(s, 0, x_t, vT, e0, rs0)
            rs1 = attn_denom(s, 1, esum1)
            attn_out(s, 1, x_t, vT, e1, rs1)
            cur = nxt


_NC_CACHE = None


def _get_nc():
    global _NC_CACHE
    if _NC_CACHE is None:
        _NC_CACHE = build()
    return _NC_CACHE


F8NP = ml_dtypes.float8_e4m3


def _tile_w_dr(w):
    """[512 out, 512 in] weight -> DoubleRow lhsT tiles [P, KP, 2, C]:
    [p, kp, i, o] = w[o, (kp*2 + i)*128 + p], scaled by W_SC, fp8."""
    wT = (W_SC * w.T).astype(np.float32)          # [c_in, o]
    return np.ascontiguousarray(
        wT.reshape(KP, 2, P, C).transpose(2, 0, 1, 3)).astype(F8NP)


def _tile_vec(v, scale=1.0):
    """[512] -> [128, 4] per-partition scalars: [p, kt] = scale*v[kt*128+p]"""
    return np.ascontiguousarray(
        (scale * v).astype(np.float32).reshape(CT, P).T)


def make_in_maps(x, gn_w, gn_b, qkv_w, qkv_b, proj_w, proj_b):
    x = np.asarray(x, dtype=np.float32)
    gn_w = np.asarray(gn_w, dtype=np.float32)
    gn_b = np.asarray(gn_b, dtype=np.float32)
    qkv_w = np.asarray(qkv_w, dtype=np.float32)
    qkv_b = np.asarray(qkv_b, dtype=np.float32)
    proj_w = np.asarray(proj_w, dtype=np.float32)
    proj_b = np.asarray(proj_b, dtype=np.float32)

    xr = x.reshape(B, C, HW)
    gmat = np.kron(np.eye(GROUPS, dtype=np.float32),
                   np.ones((C // GROUPS, 1), dtype=np.float32))  # [512, 32]
    gm_t = np.ascontiguousarray(
        gmat.reshape(CT, P, GROUPS).transpose(1, 0, 2)).astype(np.float32)
    gmT_t = np.ascontiguousarray(gmat.T).astype(np.float32)      # [32, 512]

    # fold proj into v: W' = proj_w @ wv; bias collapses to a constant
    # output offset bo = proj_w @ bv + pb (softmax rows sum to 1)
    wv_folded = proj_w @ qkv_w[2 * C:3 * C]
    bo = proj_w @ qkv_b[2 * C:3 * C] + proj_b

    common = {
        "wq": _tile_w_dr(qkv_w[0:C]),
        "wk": _tile_w_dr(qkv_w[C:2 * C]),
        "wv": _tile_w_dr(wv_folded),
        "gm": gm_t,
        "gmT": gmT_t,
        "bq": _tile_vec(qkv_b[0:C], QK_SC),
        "bk": _tile_vec(qkv_b[C:2 * C], QK_SC),
        "bo": _tile_vec(bo),
        "gnw": _tile_vec(gn_w, H_SC),
        "gnb": _tile_vec(gn_b, H_SC),
    }
    in_maps = []
    for c in range(N_CORES):
        m = dict(common)
        m["x"] = np.ascontiguousarray(xr[c * BS:(c + 1) * BS])
        in_maps.append(m)
    return in_maps


def kernel(**inputs):
    in_maps = make_in_maps(**inputs)
    nc = _get_nc()
    res = run_bass_kernel_spmd(nc, in_maps, core_ids=list(range(N_CORES)))
    out = np.concatenate([res.results[c]["out"] for c in range(N_CORES)],
                         axis=0)
    return out.reshape(B, C, H, W).astype(np.float32)


# revision 10
# speedup vs baseline: 1.0541x; 1.0541x over previous
"""DiffAE attention block (GroupNorm -> qkv 1x1conv -> attention -> proj -> residual)
as a Bass/Tile kernel on 8 TRN2 NeuronCores.

Sharding: data-parallel over batch. B=32 samples, 4 per core. Attention is
per-sample, so no collectives are needed: inputs are sharded host-side and
outputs gathered host-side.

Math restructure vs the straightforward reference:
  * proj is folded into the v weights host-side: W' = proj_w @ wv, and since
    the per-column softmax scale commutes with the channel projection and
    softmax rows sum to exactly 1 against the kernel's own denominator, the
    v/proj biases collapse to a constant output bias bo = proj_w @ bv + pb.
    This removes the whole proj matmul stage (12.5% of the FLOPs).
  * All four remaining matmul stages (q, k, scores, attn@v) run in fp8-e4m3
    with perf_mode=DoubleRow (K=256 per instruction, measured ~259ns per
    [K=256]x[128,2x512] instruction = ~1.6x bf16 TensorE FLOP rate).
    fp32 accumulation in PSUM throughout.
  * fp8 scaling: TRN e4m3 spans [2^-9, 240]. GroupNorm output h is stored
    x8, q/k weights x4 (so q,k tiles are 32x), W' x4 (v tiles 32x). Scores
    PSUM = 1024 x true score; exp applies scale SCALE/1024 and bias -2 so the
    largest exp value stays ~40 << 240 (softmax shift-invariance cancels the
    -2 against the denominator). The 32.0-matrix for the denominator
    partition-broadcast makes rs = 1/(32*sum e) cancel the 32x in v.

Engine assignment (engine-balance, measured rates DVE 0.96GHz, ACT 1.2GHz,
Pool ~0.5-1.0x 1.2GHz, exp must be ACT):
  ACT   : exp (only table func -> no ACT table swaps), GroupNorm affine
  DVE   : q/k PSUM evict + bias, bn_stats/aggr, Newton rsqrt (no ACT Sqrt!),
          fast reciprocal, h2*rs
  Pool  : v PSUM evict (copy), final (t + bo + x) output op
  PE    : everything matmul, incl. softmax denominator via 32.0-matmul
          DoubleRow accumulation over the e super-tile
"""

import numpy as np
import ml_dtypes

import concourse.bacc as bacc
import concourse.bass as bass
import concourse.mybir as mybir
import concourse.tile as tile
from concourse import bass_isa
from concourse.bass_utils import run_bass_kernel_spmd

N_CORES = 8
B, C, H, W = 32, 512, 32, 32
HW = H * W                      # 1024 spatial positions
BS = B // N_CORES               # 4 samples per core
GROUPS = 32
EPS = 1e-5
SCALE = float(C) ** -0.5
P = 128
CT = C // P                     # 4 channel tiles
MT = HW // P                    # 8 spatial tiles
KP = CT // 2                    # 2 DoubleRow contraction pairs over channels
MP = MT // 2                    # 4 DoubleRow contraction pairs over spatial
NF = 512                        # matmul moving-dim chunk (output columns)
NCH = HW // NF                  # 2 column chunks
F32 = mybir.dt.float32
F32R = mybir.dt.float32r
I32 = mybir.dt.int32
BF16 = mybir.dt.bfloat16
F8 = mybir.dt.float8e4
AX = mybir.AxisListType
ALU = mybir.AluOpType
ACTF = mybir.ActivationFunctionType
DR = mybir.MatmulPerfMode.DoubleRow

H_SC = 8.0                      # h stored as 8*h
W_SC = 4.0                      # q/k/v weights stored as 4*W
QK_SC = H_SC * W_SC             # q,k tiles are 32x true
S_SC = QK_SC * QK_SC            # scores PSUM is 1024x true
E_BIAS = -2.0                   # exp(s - 2): keeps max e ~40 << 240 (fp8 max)
ONE_V = 32.0                    # denominator matmul constant; 1/(32 sum e)
                                # cancels the 32x in the v tiles
RSQRT_MAGIC = 0x5F3759DF
RSQRT_SUB = 0x7FFFFFFF - RSQRT_MAGIC


def build():
    nc = bacc.Bacc("TRN2", target_bir_lowering=False, debug=False,
                   num_devices=N_CORES, num_swdge_queues=4)

    x_d = nc.declare_dram_parameter("x", [BS, C, HW], F32, isOutput=False)
    wq_d = nc.declare_dram_parameter("wq", [P, KP, 2, C], F8, isOutput=False)
    wk_d = nc.declare_dram_parameter("wk", [P, KP, 2, C], F8, isOutput=False)
    wv_d = nc.declare_dram_parameter("wv", [P, KP, 2, C], F8, isOutput=False)
    gm_d = nc.declare_dram_parameter("gm", [P, CT, GROUPS], F32R, isOutput=False)
    gmT_d = nc.declare_dram_parameter("gmT", [GROUPS, C], F32R, isOutput=False)
    bq_d = nc.declare_dram_parameter("bq", [P, CT], F32, isOutput=False)
    bk_d = nc.declare_dram_parameter("bk", [P, CT], F32, isOutput=False)
    bo_d = nc.declare_dram_parameter("bo", [P, CT], F32, isOutput=False)
    gnw_d = nc.declare_dram_parameter("gnw", [P, CT], F32, isOutput=False)
    gnb_d = nc.declare_dram_parameter("gnb", [P, CT], F32, isOutput=False)
    out_d = nc.declare_dram_parameter("out", [BS, C, HW], F32, isOutput=True)

    with tile.TileContext(nc) as tc:
        build_tile(tc, x_d, wq_d, wk_d, wv_d, gm_d, gmT_d,
                   bq_d, bk_d, bo_d, gnw_d, gnb_d, out_d)
    nc.finalize()
    return nc


def build_tile(tc, x_d, wq_d, wk_d, wv_d, gm_d, gmT_d,
               bq_d, bk_d, bo_d, gnw_d, gnb_d, out_d):
    nc = tc.nc
    from contextlib import ExitStack
    with ExitStack() as ctx:
        ctx.enter_context(nc.allow_low_precision(
            reason="fp8 DoubleRow matmuls; fp32 accumulate in PSUM"))
        consts = ctx.enter_context(tc.tile_pool(name="consts", bufs=1))
        xs = ctx.enter_context(tc.tile_pool(name="xs", bufs=8))
        hp = ctx.enter_context(tc.tile_pool(name="hp", bufs=2))
        qp = ctx.enter_context(tc.tile_pool(name="qp", bufs=2))
        kp_ = ctx.enter_context(tc.tile_pool(name="kp", bufs=2))
        vp = ctx.enter_context(tc.tile_pool(name="vp", bufs=2))
        ep = ctx.enter_context(tc.tile_pool(name="ep", bufs=4))
        rp = ctx.enter_context(tc.tile_pool(name="rp", bufs=3))
        op = ctx.enter_context(tc.tile_pool(name="op", bufs=10))
        st = ctx.enter_context(tc.tile_pool(name="st", bufs=16))
        pgn = ctx.enter_context(tc.tile_pool(name="pgn", bufs=2, space="PSUM"))
        pmm = ctx.enter_context(tc.tile_pool(name="pmm", bufs=6, space="PSUM"))

        # ---- constants / weights (small ones first so the first sample's
        # GroupNorm isn't stuck behind the weight transfers) ----
        gm_sb = consts.tile([P, CT, GROUPS], F32R, name="gm_sb")
        gmT_sb = consts.tile([GROUPS, C], F32R, name="gmT_sb")
        bq_sb = consts.tile([P, CT], F32, name="bq_sb")
        bk_sb = consts.tile([P, CT], F32, name="bk_sb")
        bo_sb = consts.tile([P, CT], F32, name="bo_sb")
        gnw_sb = consts.tile([P, CT], F32, name="gnw_sb")
        gnb_sb = consts.tile([P, CT], F32, name="gnb_sb")
        epsg_sb = consts.tile([GROUPS, 1], F32, name="epsg_sb")
        ebias_sb = consts.tile([P, 1], F32, name="ebias_sb")
        ones_sb = consts.tile([P, 2, P], F8, name="ones_sb")
        wq_sb = consts.tile([P, KP, 2, C], F8, name="wq_sb")
        wk_sb = consts.tile([P, KP, 2, C], F8, name="wk_sb")
        wv_sb = consts.tile([P, KP, 2, C], F8, name="wv_sb")

        for sb, d in ((gm_sb, gm_d), (gmT_sb, gmT_d),
                      (bq_sb, bq_d), (bk_sb, bk_d), (bo_sb, bo_d),
                      (gnw_sb, gnw_d), (gnb_sb, gnb_d)):
            nc.gpsimd.dma_start(out=sb, in_=d[:])
        nc.vector.memset(epsg_sb, EPS)
        nc.vector.memset(ebias_sb, E_BIAS)
        nc.vector.memset(ones_sb, ONE_V)

        inv_gsz = 1.0 / (C // GROUPS * HW)

        def prep_load(s):
            """x DMA for sample s."""
            x_t = []
            for ct in range(CT):
                xt = xs.tile([P, HW], F32, name=f"x_s{s}_{ct}", tag="x")
                for n in range(NCH):
                    nsl = slice(n * NF, (n + 1) * NF)
                    nc.sync.dma_start(out=xt[:, nsl],
                                      in_=x_d[s, ct * P:(ct + 1) * P, nsl])
                x_t.append(xt)
            if s == 0:
                # weights go after the first x so sample 0's stats start
                # sooner; they are small in fp8 (256KB each)
                for sb, d in ((wq_sb, wq_d), (wk_sb, wk_d), (wv_sb, wv_d)):
                    nc.gpsimd.dma_start(out=sb, in_=d[:])
            return x_t

        def prep_stats(s, x_t):
            """GroupNorm for sample s -> h super-tile (fp8, 8x scaled)."""
            gsum = pgn.tile([GROUPS, 2], F32, name=f"gsum_{s}", tag="ps")
            bnsts = []
            for ct in range(CT):
                bnst = st.tile([P, NCH, 6], F32, name=f"bnst_{s}_{ct}",
                               tag=f"bnst{ct}")
                xv = x_t[ct].rearrange("p (a b) -> p a b", b=NF)
                for sg in range(NCH):
                    nc.vector.bn_stats(bnst[:, sg, :], xv[:, sg, :])
                bnsts.append(bnst)
            for ct in range(CT):
                rowmv = st.tile([P, 2], F32, name=f"rowmv_{s}_{ct}",
                                tag="rowmv")
                nc.vector.bn_aggr(rowmv, bnsts[ct])
                # me = [mean, E[x^2]] * HW (uniform scale folds into inv_gsz)
                me = st.tile([P, 2], F32R, name=f"me_{s}_{ct}", tag="me")
                nc.vector.tensor_scalar_mul(me[:, 0:1], rowmv[:, 0:1],
                                            float(HW))
                nc.vector.scalar_tensor_tensor(
                    out=me[:, 1:2], in0=rowmv[:, 0:1],
                    scalar=rowmv[:, 0:1], in1=rowmv[:, 1:2],
                    op0=ALU.mult, op1=ALU.add)
                nc.vector.tensor_scalar_mul(me[:, 1:2], me[:, 1:2],
                                            float(HW))
                nc.tensor.matmul(gsum, lhsT=gm_sb[:, ct, :], rhs=me,
                                 start=(ct == 0), stop=(ct == CT - 1))

            # mv[:,0] = mean, mv[:,1] = 1/sqrt(var+eps) via Newton rsqrt on
            # DVE (keeps Sqrt off ACT so the exp table never gets evicted)
            mv = st.tile([GROUPS, 2], F32R, name=f"mv_{s}", tag="mv")
            nc.scalar.mul(out=mv[:, 0:1], in_=gsum[:, 0:1], mul=inv_gsz)
            ex2 = st.tile([GROUPS, 1], F32, name=f"ex2_{s}", tag="ex2")
            nc.scalar.mul(out=ex2, in_=gsum[:, 1:2], mul=inv_gsz)
            msq = st.tile([GROUPS, 1], F32, name=f"msq_{s}", tag="msq")
            nc.vector.tensor_mul(msq, mv[:, 0:1], mv[:, 0:1])
            vpe = st.tile([GROUPS, 1], F32, name=f"vpe_{s}", tag="vpe")
            nc.vector.tensor_sub(vpe, ex2, msq)
            nc.vector.tensor_add(vpe, vpe, epsg_sb)
            y = st.tile([GROUPS, 1], F32, name=f"y_{s}", tag="y")
            yi = y.bitcast(I32)
            vi = vpe.bitcast(I32)
            # y0 = magic - (i >> 1), via NOT31(t) - (0x7FFFFFFF - magic)
            nc.vector.tensor_scalar(out=yi, in0=vi, scalar1=1,
                                    scalar2=0x7FFFFFFF,
                                    op0=ALU.logical_shift_right,
                                    op1=ALU.bitwise_xor)
            nc.vector.tensor_scalar(out=yi, in0=yi, scalar1=RSQRT_SUB,
                                    scalar2=None, op0=ALU.subtract)
            a = st.tile([GROUPS, 1], F32, name=f"a_{s}", tag="a")
            for it in range(2):
                nc.vector.tensor_mul(a, y, y)
                nc.vector.tensor_mul(a, a, vpe)
                nc.vector.tensor_scalar(out=a, in0=a, scalar1=-0.5,
                                        scalar2=1.5, op0=ALU.mult,
                                        op1=ALU.add)
                dst = mv[:, 1:2] if it == 1 else y
                nc.vector.tensor_mul(dst, y, a)

            # bo is folded into x in place (on Pool, which cannot take
            # per-partition scalar ops but can do a 0-stride-broadcast
            # tensor add); the GroupNorm beta is compensated by -alpha*bo
            # so h stays correct. The final output op then needs only a
            # plain (t + x') add, which Pool can run off the DVE/ACT path.
            for ct in range(CT):
                b = bo_sb[:, ct:ct + 1]
                bb = bass.AP(tensor=b.tensor, offset=b.offset,
                             ap=[[b.ap[0][0], P], [0, HW]])
                nc.gpsimd.tensor_add(x_t[ct], x_t[ct], bb)

            # expand per-group (mean, rstd) to per-channel alpha/beta;
            # gnw/gnb arrive pre-scaled by 8 so h is stored as 8*h.
            # The affine itself runs on ACT (Identity, scale/bias APs).
            ht = hp.tile([P, CT, HW], F8, name=f"h_{s}", tag="h")
            for ct in range(CT):
                eps_ps = pgn.tile([P, 2], F32, name=f"exp_{s}_{ct}", tag="ps")
                nc.tensor.matmul(eps_ps, lhsT=gmT_sb[:, ct * P:(ct + 1) * P],
                                 rhs=mv, start=True, stop=True)
                exs = st.tile([P, 2], F32, name=f"exs_{s}_{ct}", tag="exs")
                nc.vector.tensor_copy(exs, eps_ps)
                alpha = st.tile([P, 1], F32, name=f"al_{s}_{ct}", tag="al")
                nc.vector.tensor_mul(alpha, gnw_sb[:, ct:ct + 1], exs[:, 1:2])
                mpb = st.tile([P, 1], F32, name=f"mpb_{s}_{ct}", tag="mpb")
                nc.vector.tensor_add(mpb, exs[:, 0:1], bo_sb[:, ct:ct + 1])
                mal = st.tile([P, 1], F32, name=f"mal_{s}_{ct}", tag="mal")
                nc.vector.tensor_mul(mal, mpb, alpha)
                beta = st.tile([P, 1], F32, name=f"be_{s}_{ct}", tag="be")
                nc.vector.tensor_sub(beta, gnb_sb[:, ct:ct + 1], mal)
                nc.scalar.activation(out=ht[:, ct, :], in_=x_t[ct],
                                     func=ACTF.Identity,
                                     bias=beta, scale=alpha)
            return ht

        def body_qkv(s, h_t):
            """q, k (channel-major) and vT (spatial-major) for sample s."""
            q_t = qp.tile([P, CT, HW], F8, name=f"q_{s}", tag="q")
            k_t = kp_.tile([P, CT, HW], F8, name=f"k_{s}", tag="k")
            for w_sb, b_sb, dst, tag in ((wq_sb, bq_sb, q_t, "q"),
                                         (wk_sb, bk_sb, k_t, "k")):
                for mt in range(CT):
                    ps_n = [pmm.tile([P, NF], F32,
                                     name=f"{tag}p_{s}_{mt}_{n}", tag="ps")
                            for n in range(NCH)]
                    for kpi in range(KP):
                        lhsT = w_sb[:, kpi, :, mt * P:(mt + 1) * P]
                        for n in range(NCH):
                            nsl = slice(n * NF, (n + 1) * NF)
                            nc.tensor.matmul(
                                ps_n[n], lhsT=lhsT,
                                rhs=h_t[:, 2 * kpi:2 * kpi + 2, nsl],
                                start=(kpi == 0), stop=(kpi == KP - 1),
                                perf_mode=DR)
                    for n in range(NCH):
                        nsl = slice(n * NF, (n + 1) * NF)
                        nc.vector.tensor_scalar(
                            out=dst[:, mt, nsl], in0=ps_n[n],
                            scalar1=b_sb[:, mt:mt + 1], scalar2=None,
                            op0=ALU.add)

            vT = vp.tile([P, MT, C], F8, name=f"v_{s}", tag="v")
            for mt in range(MT):
                ps = pmm.tile([P, C], F32, name=f"vp_{s}_{mt}", tag="ps")
                for kpi in range(KP):
                    nc.tensor.matmul(
                        ps, lhsT=h_t[:, 2 * kpi:2 * kpi + 2,
                                     mt * P:(mt + 1) * P],
                        rhs=wv_sb[:, kpi, :, :],
                        start=(kpi == 0), stop=(kpi == KP - 1),
                        perf_mode=DR)
                nc.scalar.activation(out=vT[:, mt, :], in_=ps,
                                     func=ACTF.Copy)
            return q_t, k_t, vT

        def attn_scores(s, n, q_t, k_t):
            """scores + exp for column chunk n -> e super-tile."""
            nsl = slice(n * NF, (n + 1) * NF)
            e_t = ep.tile([P, MT, NF], F8, name=f"e_{s}_{n}", tag="e")
            for mt in range(MT):
                ps = pmm.tile([P, NF], F32, name=f"ep_{s}_{n}_{mt}", tag="ps")
                for kpi in range(KP):
                    nc.tensor.matmul(
                        ps, lhsT=k_t[:, 2 * kpi:2 * kpi + 2,
                                     mt * P:(mt + 1) * P],
                        rhs=q_t[:, 2 * kpi:2 * kpi + 2, nsl],
                        start=(kpi == 0), stop=(kpi == KP - 1),
                        perf_mode=DR)
                nc.scalar.activation(out=e_t[:, mt, :], in_=ps, func=ACTF.Exp,
                                     scale=SCALE / S_SC, bias=ebias_sb)
            return e_t

        def attn_denom(s, n, e_t):
            """denominator via 32.0-matmul DoubleRow accumulation over the
            e super-tile (partition-broadcast) + fast reciprocal -> rs."""
            sb_ps = pmm.tile([P, NF], F32, name=f"sb_{s}_{n}", tag="ps")
            for mp in range(MP):
                nc.tensor.matmul(sb_ps, lhsT=ones_sb,
                                 rhs=e_t[:, 2 * mp:2 * mp + 2, :],
                                 start=(mp == 0), stop=(mp == MP - 1),
                                 perf_mode=DR)
            rs = rp.tile([P, NF], F32, name=f"rs_{s}_{n}", tag="rs")
            nc.vector.reciprocal_approx_fast(out=rs, in_=sb_ps)
            return rs

        def attn_out(s, n, x_t, vT, e_t, rs):
            """(v' @ e^T) * rs + bo + x, store for column chunk n."""
            nsl = slice(n * NF, (n + 1) * NF)
            for ct in range(CT):
                ps = pmm.tile([P, NF], F32, name=f"h2p_{s}_{n}_{ct}",
                              tag="ps")
                for mp in range(MP):
                    nc.tensor.matmul(
                        ps, lhsT=vT[:, 2 * mp:2 * mp + 2,
                                    ct * P:(ct + 1) * P],
                        rhs=e_t[:, 2 * mp:2 * mp + 2, :],
                        start=(mp == 0), stop=(mp == MP - 1),
                        perf_mode=DR)
                t_sb = op.tile([P, NF], F32, name=f"t_{s}_{n}_{ct}", tag="t")
                nc.vector.tensor_mul(t_sb, ps, rs)
                o_sb = op.tile([P, NF], F32, name=f"o_{s}_{n}_{ct}", tag="o")
                nc.gpsimd.tensor_add(o_sb, t_sb, x_t[ct][:, nsl])
                nc.sync.dma_start(
                    out=out_d[s, ct * P:(ct + 1) * P, nsl], in_=o_sb)

        # software pipeline: sample s+1's x load + GroupNorm stats chain
        # are emitted inside sample s's body so the serial stats chain hides
        # under the qkv/attention matmul burst
        x0 = prep_load(0)
        h0 = prep_stats(0, x0)
        cur = (x0, h0)
        for s in range(BS):
            x_t, h_t = cur
            nxt_x = prep_load(s + 1) if s + 1 < BS else None
            q_t, k_t, vT = body_qkv(s, h_t)
            nxt = (nxt_x, prep_stats(s + 1, nxt_x)) if s + 1 < BS else None
            e0 = attn_scores(s, 0, q_t, k_t)
            e1 = attn_scores(s, 1, q_t, k_t)
            rs0 = attn_denom(s, 0, e0)
            attn_out(s, 0, x_t, vT, e0, rs0)
            rs1 = attn_denom(s, 1, e1)
            attn_out(s, 1, x_t, vT, e1, rs1)
            cur = nxt


_NC_CACHE = None


def _get_nc():
    global _NC_CACHE
    if _NC_CACHE is None:
        _NC_CACHE = build()
    return _NC_CACHE


F8NP = ml_dtypes.float8_e4m3


def _tile_w_dr(w):
    """[512 out, 512 in] weight -> DoubleRow lhsT tiles [P, KP, 2, C]:
    [p, kp, i, o] = w[o, (kp*2 + i)*128 + p], scaled by W_SC, fp8."""
    wT = (W_SC * w.T).astype(np.float32)          # [c_in, o]
    return np.ascontiguousarray(
        wT.reshape(KP, 2, P, C).transpose(2, 0, 1, 3)).astype(F8NP)


def _tile_vec(v, scale=1.0):
    """[512] -> [128, 4] per-partition scalars: [p, kt] = scale*v[kt*128+p]"""
    return np.ascontiguousarray(
        (scale * np.asarray(v, dtype=np.float64)).astype(np.float32)
        .reshape(CT, P).T)


def make_in_maps(x, gn_w, gn_b, qkv_w, qkv_b, proj_w, proj_b):
    x = np.asarray(x, dtype=np.float32)
    gn_w = np.asarray(gn_w, dtype=np.float32)
    gn_b = np.asarray(gn_b, dtype=np.float32)
    qkv_w = np.asarray(qkv_w, dtype=np.float32)
    qkv_b = np.asarray(qkv_b, dtype=np.float32)
    proj_w = np.asarray(proj_w, dtype=np.float32)
    proj_b = np.asarray(proj_b, dtype=np.float32)

    xr = x.reshape(B, C, HW)
    gmat = np.kron(np.eye(GROUPS, dtype=np.float32),
                   np.ones((C // GROUPS, 1), dtype=np.float32))  # [512, 32]
    gm_t = np.ascontiguousarray(
        gmat.reshape(CT, P, GROUPS).transpose(1, 0, 2)).astype(np.float32)
    gmT_t = np.ascontiguousarray(gmat.T).astype(np.float32)      # [32, 512]

    # fold proj into v: W' = proj_w @ wv; bias collapses to a constant
    # output offset bo = proj_w @ bv + pb (softmax rows sum to 1)
    wv_folded = proj_w @ qkv_w[2 * C:3 * C]
    bo = proj_w @ qkv_b[2 * C:3 * C] + proj_b

    common = {
        "wq": _tile_w_dr(qkv_w[0:C]),
        "wk": _tile_w_dr(qkv_w[C:2 * C]),
        "wv": _tile_w_dr(wv_folded),
        "gm": gm_t,
        "gmT": gmT_t,
        "bq": _tile_vec(qkv_b[0:C], QK_SC),
        "bk": _tile_vec(qkv_b[C:2 * C], QK_SC),
        "bo": _tile_vec(bo),
        "gnw": _tile_vec(gn_w, H_SC),
        "gnb": _tile_vec(gn_b, H_SC),
    }
    in_maps = []
    for c in range(N_CORES):
        m = dict(common)
        m["x"] = np.ascontiguousarray(xr[c * BS:(c + 1) * BS])
        in_maps.append(m)
    return in_maps


def kernel(**inputs):
    in_maps = make_in_maps(**inputs)
    nc = _get_nc()
    res = run_bass_kernel_spmd(nc, in_maps, core_ids=list(range(N_CORES)))
    out = np.concatenate([res.results[c]["out"] for c in range(N_CORES)],
                         axis=0)
    return out.reshape(B, C, H, W).astype(np.float32)
